# revision 31
# baseline (speedup 1.0000x reference)
"""Trainium2 Bass kernel: GPT-2-style causal multi-head attention.

Problem: B=4, S=2048, D=1024, H=16 heads (head_dim 64), fp32.
  q/k/v = x @ W{q,k,v} + b{q,k,v}; causal softmax attention; out = attn_out @ Wo + bo.

Sharding (8 cores): core c owns batch b = c//2 and head-group g = c%2
(8 heads = 512 feature dims). Wq/Wk/Wv column-sliced, Wo row-sliced per core.
Each core emits a partial o_proj output out_t [D, S] (transposed); the host
sums the pair of partials per batch, transposes, and adds the folded bias
bo' = bv @ Wo + bo (exact: softmax rows sum to 1, so attention(v + bv) =
attention(v) + bv, and the +bv term commutes through the o_proj matmul).

All matmuls run in bf16 (fp32 PSUM accumulation). x arrives pre-transposed
and pre-cast from the host as x^T [D, S] bf16, so no on-chip transposes:
  q^T/k^T = W^T-slices (stationary) x^T (moving)        [j, s] layout
  v       = x^T-slices (stationary) Wv (moving)         [s, j] natural layout
  scores  S^T[k, q] = k^T (stationary, K=64) q^T (moving) -- the two heads of
          a j-tile use partition ranges 0:64 / 64:128, so their score matmuls
          land on disjoint PE row-groups and execute concurrently.
  exp on ACT (PSUM->SBUF bf16), causal diagonal masked in-place on GpSimd.
  AV: attnout^T = v-slice+ones-col (stationary, M=65) P^T (moving); row 64
      accumulates the softmax denominator.
  normalize: reciprocal_approx_fast + K=1 ones-matmul partition-broadcast +
      DVE multiply -> aot bf16; o_proj = Wo-slices (stationary) aot (moving).
"""

import sys

sys.path.insert(0, "/opt/trn_rl_repo")

import numpy as np
import ml_dtypes

import concourse.bass as bass
import concourse.bacc as bacc
import concourse.tile as tile
import concourse.mybir as mybir
from concourse.bass_utils import run_bass_kernel_spmd

F32 = mybir.dt.float32
F32R = mybir.dt.float32r
BF16 = mybir.dt.bfloat16
BF16_NP = ml_dtypes.bfloat16

B, S, D, H = 4, 2048, 1024, 16
HD = D // H  # 64
N_CORES = 8
HPC = 8  # heads per core
J = HPC * HD  # per-core feature dims = 512
NJB = J // 128  # j-blocks per core = 4
NDB = D // 128  # d-blocks = 8
NSB = S // 128  # s-blocks = 16
NC = S // 512  # 512-chunks = 4
VW = 65  # v-tile stride per head: 64 cols + 1 ones col


def build_kernel(debug_dumps=False):
    nc = bacc.Bacc(
        "TRN2", target_bir_lowering=False, debug=False, enable_asserts=False,
        num_devices=N_CORES,
    )

    xt_d = nc.dram_tensor("xt", [D, S], BF16, kind="ExternalInput").ap()
    wq_d = nc.dram_tensor("wq", [D, J], BF16, kind="ExternalInput").ap()
    wk_d = nc.dram_tensor("wk", [D, J], BF16, kind="ExternalInput").ap()
    wv_d = nc.dram_tensor("wv", [D, J], BF16, kind="ExternalInput").ap()
    wo_d = nc.dram_tensor("wo", [J, D], BF16, kind="ExternalInput").ap()
    bq_d = nc.dram_tensor("bq", [128, NJB], F32, kind="ExternalInput").ap()
    bk_d = nc.dram_tensor("bk", [128, NJB], F32, kind="ExternalInput").ap()
    # o_proj is computed in two half-contraction passes (head-pairs 0-1 and
    # 2-3) so the first half can interleave into the attention stream; the
    # host sums the two partials (along with the cross-core pair).
    out_d = [
        nc.dram_tensor(f"out_t{h}", [D, S], F32, kind="ExternalOutput").ap()
        for h in range(2)
    ]
    dumps = None
    if debug_dumps:
        dumps = {
            name: nc.dram_tensor(name, shape, BF16, kind="ExternalOutput").ap()
            for name, shape in (
                ("d_qt", [128, NJB * S]),
                ("d_kt", [128, NJB * S]),
                ("d_vt", [128, NSB * HPC * VW]),
                ("d_aot", [128, NJB * S]),
            )
        }

    with tile.TileContext(nc) as tc:
        _emit(tc, nc, xt_d, wq_d, wk_d, wv_d, wo_d, bq_d, bk_d, out_d, dumps)

    nc.compile()
    return nc


def _emit(tc, nc, xt_d, wq_d, wk_d, wv_d, wo_d, bq_d, bk_d, out_d, dumps=None):
    from contextlib import ExitStack

    EXP = mybir.ActivationFunctionType.Exp
    IDENT = mybir.ActivationFunctionType.Identity

    ctx = ExitStack()
    with ctx:
        const = ctx.enter_context(tc.tile_pool(name="const", bufs=1))
        wpool = ctx.enter_context(tc.tile_pool(name="w", bufs=1))
        xpool = ctx.enter_context(tc.tile_pool(name="x", bufs=1))
        qkpool = ctx.enter_context(tc.tile_pool(name="qk", bufs=1))
        vpool = ctx.enter_context(tc.tile_pool(name="v", bufs=1))
        aopool = ctx.enter_context(tc.tile_pool(name="ao", bufs=1))
        ptpool = ctx.enter_context(tc.tile_pool(name="pt", bufs=4))
        nrmpool = ctx.enter_context(tc.tile_pool(name="nrm", bufs=3))
        ogpool = ctx.enter_context(tc.tile_pool(name="og", bufs=2))
        # PSUM: ps slots 3x[128,1024] = 6 banks; acc 2x[65,512] = 2 banks.
        ps = ctx.enter_context(tc.tile_pool(name="ps", bufs=3, space="PSUM"))
        ps_acc = ctx.enter_context(tc.tile_pool(name="ps_acc", bufs=2, space="PSUM"))

        # --- constants ---------------------------------------------------
        # ones columns (bf16) for the v-tile softmax-denominator cols
        ones_v = const.tile([128, NSB * HPC], BF16, tag="ones_v")
        nc.gpsimd.memset(ones_v[:], 1.0)
        # bf16 ones for the recip partition-broadcast matmul; row 64 is used
        # so its base partition matches the denominator row of the acc tiles
        ones_bc = const.tile([65, 64], BF16, tag="ones_bc")
        nc.gpsimd.memset(ones_bc[:], 1.0)
        # causal mask for diagonal 128x128 blocks of S^T[k, q]: keep q >= k
        mask_f = const.tile([128, 128], F32, tag="mask_f")
        nc.gpsimd.memset(mask_f[:], 1.0)
        nc.gpsimd.affine_select(
            mask_f[:], mask_f[:], pattern=[[1, 128]],
            compare_op=mybir.AluOpType.is_ge, fill=0.0,
            base=0, channel_multiplier=-1,
        )
        mask = const.tile([128, 128], BF16, tag="mask")
        nc.vector.tensor_copy(mask[:], mask_f[:])

        # --- weights / biases -------------------------------------------
        w_t = {}
        for name, wd in (("q", wq_d), ("k", wk_d), ("v", wv_d)):
            t = wpool.tile([128, NDB * J], BF16, tag=f"w{name}")
            nc.sync.dma_start(
                t[:].rearrange("p (a j) -> p a j", j=J),
                wd.rearrange("(a p) j -> p a j", p=128),
            )
            w_t[name] = t
        wo_t = wpool.tile([128, NJB * D], BF16, tag="wo")
        nc.sync.dma_start(
            wo_t[:].rearrange("p (a o) -> p a o", o=D),
            wo_d.rearrange("(a p) o -> p a o", p=128),
        )
        bqt = const.tile([128, NJB], F32, tag="bq")
        nc.sync.dma_start(bqt[:], bq_d)
        bkt = const.tile([128, NJB], F32, tag="bk")
        nc.sync.dma_start(bkt[:], bk_d)

        # --- x^T ---------------------------------------------------------
        xt = xpool.tile([128, NDB * S], BF16, tag="xt")
        nc.sync.dma_start(
            xt[:].rearrange("p (a s) -> p a s", s=S),
            xt_d.rearrange("(a p) s -> p a s", p=128),
        )

        # --- v natural [s, j] with ones cols (16 s-block groups) ---------
        # v_t cols: s-block sb at [sb*8*VW, ...), head h at 65h..65h+64,
        # ones at 65h+64.
        v_t = vpool.tile([128, NSB * HPC * VW], BF16, tag="vt")
        nc.vector.tensor_copy(
            v_t[:].rearrange("p (sb h c) -> p (sb h) c", sb=NSB, c=VW)[:, :, 64:65],
            ones_v[:].rearrange("p (a o) -> p a o", o=1),
        )
        for sb in range(NSB):
            pv = ps.tile([128, 1024], F32, tag="ps", name="pv")
            for db in range(NDB):
                nc.tensor.matmul(
                    pv[:, 0:J],
                    xt[:, 2048 * db + 128 * sb: 2048 * db + 128 * (sb + 1)],
                    w_t["v"][:, J * db: J * (db + 1)],
                    start=(db == 0), stop=(db == NDB - 1),
                )
            nc.scalar.activation(
                v_t[:, sb * HPC * VW: (sb + 1) * HPC * VW].rearrange(
                    "p (h c) -> p h c", c=VW)[:, :, 0:64],
                pv[:, 0:J].rearrange("p (h c) -> p h c", c=64),
                IDENT,
            )

        # --- q^T / k^T [j, s] --------------------------------------------
        q_t = qkpool.tile([128, NJB * S], BF16, tag="qt")
        k_t = qkpool.tile([128, NJB * S], BF16, tag="kt")

        def emit_qk(jb):
            for name, dst, bias in (("q", q_t, bqt), ("k", k_t, bkt)):
                for c in range(NC):
                    pq = ps.tile([128, 1024], F32, tag="ps", name="pq")
                    for db in range(NDB):
                        nc.tensor.matmul(
                            pq[:, 0:512],
                            w_t[name][:, J * db + 128 * jb: J * db + 128 * (jb + 1)],
                            xt[:, 2048 * db + 512 * c: 2048 * db + 512 * (c + 1)],
                            start=(db == 0), stop=(db == NDB - 1),
                        )
                    nc.scalar.activation(
                        dst[:, S * jb + 512 * c: S * jb + 512 * (c + 1)],
                        pq[:, 0:512],
                        IDENT,
                        bias=bias[:, jb: jb + 1],
                    )

        # --- o_proj halves: out^T[o, s] += Wo_jb^T @ aot_jb ---------------
        # Generator yields one output-block at a time so half 0 can be
        # interleaved into pairs 2-3's attention as PE gap-filler work.
        def emit_oproj_half(h):
            for ob in range(NDB):
                og = ogpool.tile([128, S], F32, tag="og")
                for c in range(NC):
                    po = ps.tile([128, 1024], F32, tag="ps", name="po")
                    for jj in range(2):
                        jb = 2 * h + jj
                        nc.tensor.matmul(
                            po[:, 0:512],
                            wo_t[:, D * jb + 128 * ob: D * jb + 128 * (ob + 1)],
                            aot[:, S * jb + 512 * c: S * jb + 512 * (c + 1)],
                            start=(jj == 0), stop=(jj == 1),
                        )
                    if c % 2 == 0:
                        nc.scalar.copy(og[:, 512 * c: 512 * (c + 1)], po[:, 0:512])
                    else:
                        nc.vector.tensor_copy(
                            og[:, 512 * c: 512 * (c + 1)], po[:, 0:512])
                nc.sync.dma_start(out_d[h][128 * ob: 128 * (ob + 1), :], og[:])
                yield

        # --- attention: 4 head-pairs (j-tiles), 4 q-chunks each ----------
        # Per (pair, chunk, kb): one combined score tile holds head A in cols
        # 0:512 and head B in 512:1024 (separate PSUM banks -> the two K=64
        # matmuls run on disjoint PE row-groups concurrently); one 2D-AP exp
        # covers both heads; AV accumulates per head into [65, 512] accs.
        aot = aopool.tile([128, NJB * S], BF16, tag="aot")
        rows = {"A": slice(0, 64), "B": slice(64, 128)}
        filler = None
        for t in range(NJB):
            # q/k for pair t+? : emitted here so pair t-1's attention has
            # lower-priority PE work available to fill its dependency gaps
            emit_qk(t)
            if t == 2:
                filler = emit_oproj_half(0)
            head = {"A": 2 * t, "B": 2 * t + 1}
            for c in range(NC):
                acc = {
                    X: ps_acc.tile([65, 512], F32, tag="acc", name=f"acc{X}")
                    for X in ("A", "B")
                }
                n_kb = 4 * c + 4

                def emit_av(pt, kb, wlo):
                    for X in ("A", "B"):
                        nc.tensor.matmul(
                            acc[X][0:65, wlo:512],
                            v_t[:, (8 * kb + head[X]) * VW:
                                (8 * kb + head[X]) * VW + VW],
                            pt[:, 512 * (X == "B") + wlo:
                               512 * (X == "B") + 512],
                            start=(kb == 0), stop=(kb == n_kb - 1),
                        )

                # software-pipelined: AV(kb-1) is emitted AFTER scores(kb) so
                # the in-order PE queue streams scores while ACT runs the exp
                # (an AV waiting on its exp would otherwise block the queue)
                pend = None
                for kb in range(n_kb):
                    wlo = max(0, 128 * kb - 512 * c)
                    st = ps.tile([128, 1024], F32, tag="ps", name="st")
                    for X in ("A", "B"):
                        nc.tensor.matmul(
                            st[:, 512 * (X == "B") + wlo:
                               512 * (X == "B") + 512],
                            k_t[rows[X],
                                S * t + 128 * kb: S * t + 128 * (kb + 1)],
                            q_t[rows[X],
                                S * t + 512 * c + wlo: S * t + 512 * (c + 1)],
                            start=True, stop=True,
                        )
                    pt = ptpool.tile([128, 1024], BF16, tag="pt", name="pt")
                    nc.scalar.activation(
                        pt[:].rearrange("p (x q) -> p x q", x=2)[:, :, wlo:512],
                        st[:].rearrange("p (x q) -> p x q", x=2)[:, :, wlo:512],
                        EXP, scale=0.125,
                    )
                    # causal mask on the diagonal 128x128 block: keep q >= k
                    if kb >= 4 * c:
                        for X in ("A", "B"):
                            o = 512 * (X == "B") + wlo
                            nc.vector.tensor_mul(
                                pt[:, o: o + 128], pt[:, o: o + 128], mask[:],
                            )
                    if pend is not None:
                        emit_av(*pend)
                    pend = (pt, kb, wlo)
                emit_av(*pend)
                # normalize: rows 0..63 raw attnout^T, row 64 = denominator.
                # Broadcast den to partitions 0..63 via a K=1 ones matmul,
                # then reciprocal at base partition 0 (reciprocal_approx_fast
                # mishandles PSUM sources and nonzero base partitions).
                # Head B lands at aot partitions 64..127 via an SBUF->SBUF
                # DMA partition shift (DVE lanes can't cross partitions).
                for X, off in (("A", 0), ("B", 64)):
                    a = acc[X]
                    dnb = nrmpool.tile([65, 512], BF16, tag="dnb")
                    nc.vector.tensor_copy(dnb[64:65, :], a[64:65, :])
                    bcd = ps.tile([64, 512], F32, tag="ps", name="bcd")
                    nc.tensor.matmul(
                        bcd[:], ones_bc[64:65, :], dnb[64:65, :],
                        start=True, stop=True,
                    )
                    dnf = nrmpool.tile([64, 512], F32, tag="dnf")
                    nc.vector.tensor_copy(dnf[:], bcd[:])
                    bct = nrmpool.tile([64, 512], F32, tag="bct")
                    nc.vector.reciprocal_approx_fast(bct[:], dnf[:])
                    if off == 0:
                        nc.vector.tensor_mul(
                            aot[0:64,
                                S * t + 512 * c: S * t + 512 * (c + 1)],
                            a[0:64, :], bct[:],
                        )
                    else:
                        tmp = nrmpool.tile([64, 512], BF16, tag="tmp")
                        nc.vector.tensor_mul(tmp[:], a[0:64, :], bct[:])
                        nc.sync.dma_start(
                            aot[64:128,
                                S * t + 512 * c: S * t + 512 * (c + 1)],
                            tmp[:],
                        )
                if filler is not None:
                    next(filler, None)

        if filler is not None:
            for _ in filler:
                pass
        for _ in emit_oproj_half(1):
            pass

        if dumps is not None:
            nc.sync.dma_start(dumps["d_qt"][:, :], q_t[:])
            nc.sync.dma_start(dumps["d_kt"][:, :], k_t[:])
            nc.sync.dma_start(dumps["d_vt"][:, :], v_t[:])
            nc.sync.dma_start(dumps["d_aot"][:, :], aot[:])

_NC_CACHE = None


def _get_nc():
    global _NC_CACHE
    if _NC_CACHE is None:
        _NC_CACHE = build_kernel()
    return _NC_CACHE


def build_in_maps(inputs):
    """Host-side sharding: per-core input dict for run_bass_kernel_spmd."""
    x = np.asarray(inputs["hidden_states"], np.float32)
    xt_b = [
        np.ascontiguousarray(x[b].T.astype(BF16_NP)) for b in range(B)
    ]  # [D, S] bf16 per batch
    Wq = np.asarray(inputs["Wq"], np.float32)
    Wk = np.asarray(inputs["Wk"], np.float32)
    Wv = np.asarray(inputs["Wv"], np.float32)
    Wo = np.asarray(inputs["Wo"], np.float32)
    bq = np.asarray(inputs["bq"], np.float32)
    bk = np.asarray(inputs["bk"], np.float32)

    in_maps = []
    for c in range(N_CORES):
        b, g = c // 2, c % 2
        js = slice(g * J, (g + 1) * J)
        in_maps.append({
            "xt": xt_b[b],
            "wq": np.ascontiguousarray(Wq[:, js].astype(BF16_NP)),
            "wk": np.ascontiguousarray(Wk[:, js].astype(BF16_NP)),
            "wv": np.ascontiguousarray(Wv[:, js].astype(BF16_NP)),
            "wo": np.ascontiguousarray(Wo[js, :].astype(BF16_NP)),
            "bq": np.ascontiguousarray(bq[js].reshape(NJB, 128).T),
            "bk": np.ascontiguousarray(bk[js].reshape(NJB, 128).T),
        })
    return in_maps


def assemble_output(results, inputs):
    """Sum per-batch partial pairs, transpose, add folded bias."""
    Wo = np.asarray(inputs["Wo"], np.float32)
    bv = np.asarray(inputs["bv"], np.float32)
    bo = np.asarray(inputs["bo"], np.float32)
    bo_f = bv @ Wo + bo
    out = np.empty((B, S, D), np.float32)
    for b in range(B):
        acc = None
        for c in (2 * b, 2 * b + 1):
            for h in range(2):
                part = results[c][f"out_t{h}"].astype(np.float32)
                acc = part if acc is None else acc + part
        out[b] = acc.T + bo_f[None, :]
    return out


def kernel(**inputs) -> np.ndarray:
    nc = _get_nc()
    in_maps = build_in_maps(inputs)
    res = run_bass_kernel_spmd(nc, in_maps, core_ids=list(range(N_CORES)))
    return assemble_output(res.results, inputs)


if __name__ == "__main__":
    rng = np.random.default_rng(0)
    ins = {
        "hidden_states": rng.standard_normal((B, S, D)).astype(np.float32),
        "Wq": (rng.standard_normal((D, D)) * 0.02).astype(np.float32),
        "bq": np.zeros(D, np.float32),
        "Wk": (rng.standard_normal((D, D)) * 0.02).astype(np.float32),
        "bk": np.zeros(D, np.float32),
        "Wv": (rng.standard_normal((D, D)) * 0.02).astype(np.float32),
        "bv": np.zeros(D, np.float32),
        "Wo": (rng.standard_normal((D, D)) * 0.02).astype(np.float32),
        "bo": np.zeros(D, np.float32),
    }
    out = kernel(**ins)
    print("out", out.shape, out.dtype, float(np.abs(out).mean()))


# revision 34
# speedup vs baseline: 1.0157x; 1.0157x over previous
"""Trainium2 Bass kernel: GPT-2-style causal multi-head attention.

Problem: B=4, S=2048, D=1024, H=16 heads (head_dim 64), fp32.
  q/k/v = x @ W{q,k,v} + b{q,k,v}; causal softmax attention; out = attn_out @ Wo + bo.

Sharding (8 cores): core c owns batch b = c//2 and head-group g = c%2
(8 heads = 512 feature dims). Wq/Wk/Wv column-sliced, Wo row-sliced per core.
Each core emits a partial o_proj output out_t [D, S] (transposed); the host
sums the pair of partials per batch, transposes, and adds the folded bias
bo' = bv @ Wo + bo (exact: softmax rows sum to 1, so attention(v + bv) =
attention(v) + bv, and the +bv term commutes through the o_proj matmul).

All matmuls run in bf16 (fp32 PSUM accumulation). x arrives pre-transposed
and pre-cast from the host as x^T [D, S] bf16, so no on-chip transposes:
  q^T/k^T = W^T-slices (stationary) x^T (moving)        [j, s] layout
  v       = x^T-slices (stationary) Wv (moving)         [s, j] natural layout
  scores  S^T[k, q] = k^T (stationary, K=64) q^T (moving) -- the two heads of
          a j-tile use partition ranges 0:64 / 64:128, so their score matmuls
          land on disjoint PE row-groups and execute concurrently.
  exp on ACT (PSUM->SBUF bf16), causal diagonal masked in-place on GpSimd.
  AV: attnout^T = v-slice+ones-col (stationary, M=65) P^T (moving); row 64
      accumulates the softmax denominator.
  normalize: reciprocal_approx_fast + K=1 ones-matmul partition-broadcast +
      DVE multiply -> aot bf16; o_proj = Wo-slices (stationary) aot (moving).
"""

import sys

sys.path.insert(0, "/opt/trn_rl_repo")

import numpy as np
import ml_dtypes

import concourse.bass as bass
import concourse.bacc as bacc
import concourse.tile as tile
import concourse.mybir as mybir
from concourse.bass_utils import run_bass_kernel_spmd

F32 = mybir.dt.float32
F32R = mybir.dt.float32r
BF16 = mybir.dt.bfloat16
BF16_NP = ml_dtypes.bfloat16

B, S, D, H = 4, 2048, 1024, 16
HD = D // H  # 64
N_CORES = 8
HPC = 8  # heads per core
J = HPC * HD  # per-core feature dims = 512
NJB = J // 128  # j-blocks per core = 4
NDB = D // 128  # d-blocks = 8
NSB = S // 128  # s-blocks = 16
NC = S // 512  # 512-chunks = 4
VW = 65  # v-tile stride per head: 64 cols + 1 ones col


def build_kernel(debug_dumps=False):
    nc = bacc.Bacc(
        "TRN2", target_bir_lowering=False, debug=False, enable_asserts=False,
        num_devices=N_CORES,
    )

    xt_d = nc.dram_tensor("xt", [D, S], BF16, kind="ExternalInput").ap()
    wq_d = nc.dram_tensor("wq", [D, J], BF16, kind="ExternalInput").ap()
    wk_d = nc.dram_tensor("wk", [D, J], BF16, kind="ExternalInput").ap()
    wv_d = nc.dram_tensor("wv", [D, J], BF16, kind="ExternalInput").ap()
    wo_d = nc.dram_tensor("wo", [J, D], BF16, kind="ExternalInput").ap()
    bq_d = nc.dram_tensor("bq", [128, NJB], F32, kind="ExternalInput").ap()
    bk_d = nc.dram_tensor("bk", [128, NJB], F32, kind="ExternalInput").ap()
    out_d = nc.dram_tensor("out_t", [D, S], F32, kind="ExternalOutput").ap()
    dumps = None
    if debug_dumps:
        dumps = {
            name: nc.dram_tensor(name, shape, BF16, kind="ExternalOutput").ap()
            for name, shape in (
                ("d_qt", [128, NJB * S]),
                ("d_kt", [128, NJB * S]),
                ("d_vt", [128, NSB * HPC * VW]),
                ("d_aot", [128, NJB * S]),
            )
        }

    with tile.TileContext(nc) as tc:
        _emit(tc, nc, xt_d, wq_d, wk_d, wv_d, wo_d, bq_d, bk_d, out_d, dumps)

    nc.compile()
    return nc


def _emit(tc, nc, xt_d, wq_d, wk_d, wv_d, wo_d, bq_d, bk_d, out_d, dumps=None):
    from contextlib import ExitStack

    EXP = mybir.ActivationFunctionType.Exp
    IDENT = mybir.ActivationFunctionType.Identity

    ctx = ExitStack()
    with ctx:
        const = ctx.enter_context(tc.tile_pool(name="const", bufs=1))
        wpool = ctx.enter_context(tc.tile_pool(name="w", bufs=1))
        xpool = ctx.enter_context(tc.tile_pool(name="x", bufs=1))
        qkpool = ctx.enter_context(tc.tile_pool(name="qk", bufs=1))
        vpool = ctx.enter_context(tc.tile_pool(name="v", bufs=1))
        aopool = ctx.enter_context(tc.tile_pool(name="ao", bufs=1))
        ptpool = ctx.enter_context(tc.tile_pool(name="pt", bufs=6))
        nrmpool = ctx.enter_context(tc.tile_pool(name="nrm", bufs=4))
        ogpool = ctx.enter_context(tc.tile_pool(name="og", bufs=2))
        # PSUM: ps slots 2x[128,1024] = 4 banks; acc 4x[65,512] = 4 banks.
        # acc=4 lets chunk c+1's AV accumulation start while chunk c's
        # normalize chain is still draining its two acc banks.
        ps = ctx.enter_context(tc.tile_pool(name="ps", bufs=2, space="PSUM"))
        ps_acc = ctx.enter_context(tc.tile_pool(name="ps_acc", bufs=4, space="PSUM"))

        # --- constants ---------------------------------------------------
        # ones columns (bf16) for the v-tile softmax-denominator cols
        ones_v = const.tile([128, NSB * HPC], BF16, tag="ones_v")
        nc.gpsimd.memset(ones_v[:], 1.0)
        # bf16 ones for the recip partition-broadcast matmul; row 64 is used
        # so its base partition matches the denominator row of the acc tiles
        ones_bc = const.tile([65, 64], BF16, tag="ones_bc")
        nc.gpsimd.memset(ones_bc[:], 1.0)
        # causal mask for diagonal 128x128 blocks of S^T[k, q]: keep q >= k
        mask_f = const.tile([128, 128], F32, tag="mask_f")
        nc.gpsimd.memset(mask_f[:], 1.0)
        nc.gpsimd.affine_select(
            mask_f[:], mask_f[:], pattern=[[1, 128]],
            compare_op=mybir.AluOpType.is_ge, fill=0.0,
            base=0, channel_multiplier=-1,
        )
        mask = const.tile([128, 128], BF16, tag="mask")
        nc.vector.tensor_copy(mask[:], mask_f[:])

        # --- x^T first (the v matmuls need it before anything else) ------
        xt = xpool.tile([128, NDB * S], BF16, tag="xt")
        nc.sync.dma_start(
            xt[:].rearrange("p (a s) -> p a s", s=S),
            xt_d.rearrange("(a p) s -> p a s", p=128),
        )

        # --- weights / biases (v first; Wo last, only o_proj needs it) ---
        w_t = {}
        for name, wd in (("v", wv_d), ("q", wq_d), ("k", wk_d)):
            t = wpool.tile([128, NDB * J], BF16, tag=f"w{name}")
            nc.sync.dma_start(
                t[:].rearrange("p (a j) -> p a j", j=J),
                wd.rearrange("(a p) j -> p a j", p=128),
            )
            w_t[name] = t
        bqt = const.tile([128, NJB], F32, tag="bq")
        nc.sync.dma_start(bqt[:], bq_d)
        bkt = const.tile([128, NJB], F32, tag="bk")
        nc.sync.dma_start(bkt[:], bk_d)
        wo_t = wpool.tile([128, NJB * D], BF16, tag="wo")
        nc.sync.dma_start(
            wo_t[:].rearrange("p (a o) -> p a o", o=D),
            wo_d.rearrange("(a p) o -> p a o", p=128),
        )

        # --- v natural [s, j] with ones cols (16 s-block groups) ---------
        # v_t cols: s-block sb at [sb*8*VW, ...), head h at 65h..65h+64,
        # ones at 65h+64.
        v_t = vpool.tile([128, NSB * HPC * VW], BF16, tag="vt")
        nc.vector.tensor_copy(
            v_t[:].rearrange("p (sb h c) -> p (sb h) c", sb=NSB, c=VW)[:, :, 64:65],
            ones_v[:].rearrange("p (a o) -> p a o", o=1),
        )
        for sb in range(NSB):
            pv = ps.tile([128, 1024], F32, tag="ps", name="pv")
            for db in range(NDB):
                nc.tensor.matmul(
                    pv[:, 0:J],
                    xt[:, 2048 * db + 128 * sb: 2048 * db + 128 * (sb + 1)],
                    w_t["v"][:, J * db: J * (db + 1)],
                    start=(db == 0), stop=(db == NDB - 1),
                )
            nc.scalar.activation(
                v_t[:, sb * HPC * VW: (sb + 1) * HPC * VW].rearrange(
                    "p (h c) -> p h c", c=VW)[:, :, 0:64],
                pv[:, 0:J].rearrange("p (h c) -> p h c", c=64),
                IDENT,
            )

        # --- q^T / k^T [j, s] --------------------------------------------
        q_t = qkpool.tile([128, NJB * S], BF16, tag="qt")
        k_t = qkpool.tile([128, NJB * S], BF16, tag="kt")
        for jb in range(NJB):
            for name, dst, bias in (("q", q_t, bqt), ("k", k_t, bkt)):
                for c in range(NC):
                    pq = ps.tile([128, 1024], F32, tag="ps", name="pq")
                    for db in range(NDB):
                        nc.tensor.matmul(
                            pq[:, 0:512],
                            w_t[name][:, J * db + 128 * jb: J * db + 128 * (jb + 1)],
                            xt[:, 2048 * db + 512 * c: 2048 * db + 512 * (c + 1)],
                            start=(db == 0), stop=(db == NDB - 1),
                        )
                    nc.scalar.activation(
                        dst[:, S * jb + 512 * c: S * jb + 512 * (c + 1)],
                        pq[:, 0:512],
                        IDENT,
                        bias=bias[:, jb: jb + 1],
                    )

        # --- attention: 4 head-pairs (j-tiles), 4 q-chunks each ----------
        # Per (pair, chunk, kb): one combined score tile holds head A in cols
        # 0:512 and head B in 512:1024 (separate PSUM banks -> the two K=64
        # matmuls run on disjoint PE row-groups concurrently); one 2D-AP exp
        # covers both heads; AV accumulates per head into [65, 512] accs.
        aot = aopool.tile([128, NJB * S], BF16, tag="aot")
        rows = {"A": slice(0, 64), "B": slice(64, 128)}
        for t in range(NJB):
            head = {"A": 2 * t, "B": 2 * t + 1}
            for c in range(NC):
                acc = {
                    X: ps_acc.tile([65, 512], F32, tag="acc", name=f"acc{X}")
                    for X in ("A", "B")
                }
                n_kb = 4 * c + 4

                def emit_av(pt, kb, wlo):
                    for X in ("A", "B"):
                        nc.tensor.matmul(
                            acc[X][0:65, wlo:512],
                            v_t[:, (8 * kb + head[X]) * VW:
                                (8 * kb + head[X]) * VW + VW],
                            pt[:, 512 * (X == "B") + wlo:
                               512 * (X == "B") + 512],
                            start=(kb == 0), stop=(kb == n_kb - 1),
                        )

                # software-pipelined: AV(kb-1) is emitted AFTER scores(kb) so
                # the in-order PE queue streams scores while ACT runs the exp
                # (an AV waiting on its exp would otherwise block the queue)
                pend = None
                for kb in range(n_kb):
                    wlo = max(0, 128 * kb - 512 * c)
                    st = ps.tile([128, 1024], F32, tag="ps", name="st")
                    for X in ("A", "B"):
                        nc.tensor.matmul(
                            st[:, 512 * (X == "B") + wlo:
                               512 * (X == "B") + 512],
                            k_t[rows[X],
                                S * t + 128 * kb: S * t + 128 * (kb + 1)],
                            q_t[rows[X],
                                S * t + 512 * c + wlo: S * t + 512 * (c + 1)],
                            start=True, stop=True,
                        )
                    pt = ptpool.tile([128, 1024], BF16, tag="pt", name="pt")
                    nc.scalar.activation(
                        pt[:].rearrange("p (x q) -> p x q", x=2)[:, :, wlo:512],
                        st[:].rearrange("p (x q) -> p x q", x=2)[:, :, wlo:512],
                        EXP, scale=0.125,
                    )
                    # causal mask on the diagonal 128x128 block: keep q >= k
                    if kb >= 4 * c:
                        for X in ("A", "B"):
                            o = 512 * (X == "B") + wlo
                            nc.vector.tensor_mul(
                                pt[:, o: o + 128], pt[:, o: o + 128], mask[:],
                            )
                    if pend is not None:
                        emit_av(*pend)
                    pend = (pt, kb, wlo)
                emit_av(*pend)
                # normalize: rows 0..63 raw attnout^T, row 64 = denominator.
                # Broadcast den to partitions 0..63 via a K=1 ones matmul,
                # then reciprocal at base partition 0 (reciprocal_approx_fast
                # mishandles PSUM sources and nonzero base partitions).
                # Head B lands at aot partitions 64..127 via an SBUF->SBUF
                # DMA partition shift (DVE lanes can't cross partitions).
                for X, off in (("A", 0), ("B", 64)):
                    a = acc[X]
                    dnb = nrmpool.tile([65, 512], BF16, tag="dnb")
                    nc.vector.tensor_copy(dnb[64:65, :], a[64:65, :])
                    bcd = ps.tile([64, 512], F32, tag="ps", name="bcd")
                    nc.tensor.matmul(
                        bcd[:], ones_bc[64:65, :], dnb[64:65, :],
                        start=True, stop=True,
                    )
                    dnf = nrmpool.tile([64, 512], F32, tag="dnf")
                    nc.vector.tensor_copy(dnf[:], bcd[:])
                    bct = nrmpool.tile([64, 512], F32, tag="bct")
                    nc.vector.reciprocal_approx_fast(bct[:], dnf[:])
                    if off == 0:
                        nc.vector.tensor_mul(
                            aot[0:64,
                                S * t + 512 * c: S * t + 512 * (c + 1)],
                            a[0:64, :], bct[:],
                        )
                    else:
                        tmp = nrmpool.tile([64, 512], BF16, tag="tmp")
                        nc.vector.tensor_mul(tmp[:], a[0:64, :], bct[:])
                        nc.sync.dma_start(
                            aot[64:128,
                                S * t + 512 * c: S * t + 512 * (c + 1)],
                            tmp[:],
                        )

        if dumps is not None:
            nc.sync.dma_start(dumps["d_qt"][:, :], q_t[:])
            nc.sync.dma_start(dumps["d_kt"][:, :], k_t[:])
            nc.sync.dma_start(dumps["d_vt"][:, :], v_t[:])
            nc.sync.dma_start(dumps["d_aot"][:, :], aot[:])

        # --- o_proj: out^T[o, s] partial = Wo_slice^T @ aot ---------------
        for ob in range(NDB):
            og = ogpool.tile([128, S], F32, tag="og")
            for c in range(NC):
                po = ps.tile([128, 1024], F32, tag="ps", name="po")
                for jb in range(NJB):
                    nc.tensor.matmul(
                        po[:, 0:512],
                        wo_t[:, D * jb + 128 * ob: D * jb + 128 * (ob + 1)],
                        aot[:, S * jb + 512 * c: S * jb + 512 * (c + 1)],
                        start=(jb == 0), stop=(jb == NJB - 1),
                    )
                if c % 2 == 0:
                    nc.scalar.copy(og[:, 512 * c: 512 * (c + 1)], po[:, 0:512])
                else:
                    nc.vector.tensor_copy(og[:, 512 * c: 512 * (c + 1)], po[:, 0:512])
            nc.sync.dma_start(out_d[128 * ob: 128 * (ob + 1), :], og[:])


_NC_CACHE = None


def _get_nc():
    global _NC_CACHE
    if _NC_CACHE is None:
        _NC_CACHE = build_kernel()
    return _NC_CACHE


def build_in_maps(inputs):
    """Host-side sharding: per-core input dict for run_bass_kernel_spmd."""
    x = np.asarray(inputs["hidden_states"], np.float32)
    xt_b = [
        np.ascontiguousarray(x[b].T.astype(BF16_NP)) for b in range(B)
    ]  # [D, S] bf16 per batch
    Wq = np.asarray(inputs["Wq"], np.float32)
    Wk = np.asarray(inputs["Wk"], np.float32)
    Wv = np.asarray(inputs["Wv"], np.float32)
    Wo = np.asarray(inputs["Wo"], np.float32)
    bq = np.asarray(inputs["bq"], np.float32)
    bk = np.asarray(inputs["bk"], np.float32)

    in_maps = []
    for c in range(N_CORES):
        b, g = c // 2, c % 2
        js = slice(g * J, (g + 1) * J)
        in_maps.append({
            "xt": xt_b[b],
            "wq": np.ascontiguousarray(Wq[:, js].astype(BF16_NP)),
            "wk": np.ascontiguousarray(Wk[:, js].astype(BF16_NP)),
            "wv": np.ascontiguousarray(Wv[:, js].astype(BF16_NP)),
            "wo": np.ascontiguousarray(Wo[js, :].astype(BF16_NP)),
            "bq": np.ascontiguousarray(bq[js].reshape(NJB, 128).T),
            "bk": np.ascontiguousarray(bk[js].reshape(NJB, 128).T),
        })
    return in_maps


def assemble_output(results, inputs):
    """Sum per-batch partial pairs, transpose, add folded bias."""
    Wo = np.asarray(inputs["Wo"], np.float32)
    bv = np.asarray(inputs["bv"], np.float32)
    bo = np.asarray(inputs["bo"], np.float32)
    bo_f = bv @ Wo + bo
    out = np.empty((B, S, D), np.float32)
    for b in range(B):
        acc = results[2 * b]["out_t"].astype(np.float32) + \
            results[2 * b + 1]["out_t"].astype(np.float32)
        out[b] = acc.T + bo_f[None, :]
    return out


def kernel(**inputs) -> np.ndarray:
    nc = _get_nc()
    in_maps = build_in_maps(inputs)
    res = run_bass_kernel_spmd(nc, in_maps, core_ids=list(range(N_CORES)))
    return assemble_output(res.results, inputs)


if __name__ == "__main__":
    rng = np.random.default_rng(0)
    ins = {
        "hidden_states": rng.standard_normal((B, S, D)).astype(np.float32),
        "Wq": (rng.standard_normal((D, D)) * 0.02).astype(np.float32),
        "bq": np.zeros(D, np.float32),
        "Wk": (rng.standard_normal((D, D)) * 0.02).astype(np.float32),
        "bk": np.zeros(D, np.float32),
        "Wv": (rng.standard_normal((D, D)) * 0.02).astype(np.float32),
        "bv": np.zeros(D, np.float32),
        "Wo": (rng.standard_normal((D, D)) * 0.02).astype(np.float32),
        "bo": np.zeros(D, np.float32),
    }
    out = kernel(**ins)
    print("out", out.shape, out.dtype, float(np.abs(out).mean()))


# revision 35
# speedup vs baseline: 1.0939x; 1.0770x over previous
"""Trainium2 Bass kernel: GPT-2-style causal multi-head attention.

Problem: B=4, S=2048, D=1024, H=16 heads (head_dim 64), fp32.
  q/k/v = x @ W{q,k,v} + b{q,k,v}; causal softmax attention; out = attn_out @ Wo + bo.

Sharding (8 cores): core c owns batch b = c//2 and head-group g = c%2
(8 heads = 512 feature dims). Wq/Wk/Wv column-sliced, Wo row-sliced per core.
Each core emits a partial o_proj output out_t [D, S] (transposed); the host
sums the pair of partials per batch, transposes, and adds the folded bias
bo' = bv @ Wo + bo (exact: softmax rows sum to 1, so attention(v + bv) =
attention(v) + bv, and the +bv term commutes through the o_proj matmul).

All matmuls run in bf16 (fp32 PSUM accumulation). x arrives pre-transposed
and pre-cast from the host as x^T [D, S] bf16, so no on-chip transposes:
  q^T/k^T = W^T-slices (stationary) x^T (moving)        [j, s] layout
  v       = x^T-slices (stationary) Wv (moving)         [s, j] natural layout
  scores  S^T[k, q] = k^T (stationary, K=64) q^T (moving) -- the two heads of
          a j-tile use partition ranges 0:64 / 64:128, so their score matmuls
          land on disjoint PE row-groups and execute concurrently.
  exp on ACT (PSUM->SBUF bf16), causal diagonal masked in-place on GpSimd.
  AV: attnout^T = v-slice+ones-col (stationary, M=65) P^T (moving); row 64
      accumulates the softmax denominator.
  normalize: reciprocal_approx_fast + K=1 ones-matmul partition-broadcast +
      DVE multiply -> aot bf16; o_proj = Wo-slices (stationary) aot (moving).
"""

import sys

sys.path.insert(0, "/opt/trn_rl_repo")

import numpy as np
import ml_dtypes

import concourse.bass as bass
import concourse.bacc as bacc
import concourse.tile as tile
import concourse.mybir as mybir
from concourse.bass_utils import run_bass_kernel_spmd

F32 = mybir.dt.float32
F32R = mybir.dt.float32r
BF16 = mybir.dt.bfloat16
BF16_NP = ml_dtypes.bfloat16

B, S, D, H = 4, 2048, 1024, 16
HD = D // H  # 64
N_CORES = 8
HPC = 8  # heads per core
J = HPC * HD  # per-core feature dims = 512
NJB = J // 128  # j-blocks per core = 4
NDB = D // 128  # d-blocks = 8
NSB = S // 128  # s-blocks = 16
NC = S // 512  # 512-chunks = 4
VW = 65  # v-tile stride per head: 64 cols + 1 ones col


def build_kernel(debug_dumps=False):
    nc = bacc.Bacc(
        "TRN2", target_bir_lowering=False, debug=False, enable_asserts=False,
        num_devices=N_CORES,
    )

    xt_d = nc.dram_tensor("xt", [D, S], BF16, kind="ExternalInput").ap()
    wq_d = nc.dram_tensor("wq", [D, J], BF16, kind="ExternalInput").ap()
    wk_d = nc.dram_tensor("wk", [D, J], BF16, kind="ExternalInput").ap()
    wv_d = nc.dram_tensor("wv", [D, J], BF16, kind="ExternalInput").ap()
    wo_d = nc.dram_tensor("wo", [J, D], BF16, kind="ExternalInput").ap()
    bq_d = nc.dram_tensor("bq", [128, NJB], F32, kind="ExternalInput").ap()
    bk_d = nc.dram_tensor("bk", [128, NJB], F32, kind="ExternalInput").ap()
    out_d = nc.dram_tensor("out_t", [D, S], F32, kind="ExternalOutput").ap()
    dumps = None
    if debug_dumps:
        dumps = {
            name: nc.dram_tensor(name, shape, BF16, kind="ExternalOutput").ap()
            for name, shape in (
                ("d_qt", [128, NJB * S]),
                ("d_kt", [128, NJB * S]),
                ("d_vt", [128, NSB * HPC * VW]),
                ("d_aot", [128, NJB * S]),
            )
        }

    with tile.TileContext(nc) as tc:
        _emit(tc, nc, xt_d, wq_d, wk_d, wv_d, wo_d, bq_d, bk_d, out_d, dumps)

    nc.compile()
    return nc


def _emit(tc, nc, xt_d, wq_d, wk_d, wv_d, wo_d, bq_d, bk_d, out_d, dumps=None):
    from contextlib import ExitStack

    EXP = mybir.ActivationFunctionType.Exp
    IDENT = mybir.ActivationFunctionType.Identity

    ctx = ExitStack()
    with ctx:
        const = ctx.enter_context(tc.tile_pool(name="const", bufs=1))
        wpool = ctx.enter_context(tc.tile_pool(name="w", bufs=1))
        xpool = ctx.enter_context(tc.tile_pool(name="x", bufs=1))
        qkpool = ctx.enter_context(tc.tile_pool(name="qk", bufs=1))
        vpool = ctx.enter_context(tc.tile_pool(name="v", bufs=1))
        aopool = ctx.enter_context(tc.tile_pool(name="ao", bufs=1))
        ptpool = ctx.enter_context(tc.tile_pool(name="pt", bufs=6))
        nrmpool = ctx.enter_context(tc.tile_pool(name="nrm", bufs=4))
        ogpool = ctx.enter_context(tc.tile_pool(name="og", bufs=2))
        # PSUM: ps slots 3x[128,1024] = 6 banks; acc 2x[65,512] = 2 banks.
        # (3-deep score lookahead measured faster than 2+deeper acc pool.)
        ps = ctx.enter_context(tc.tile_pool(name="ps", bufs=3, space="PSUM"))
        ps_acc = ctx.enter_context(tc.tile_pool(name="ps_acc", bufs=2, space="PSUM"))

        # --- constants ---------------------------------------------------
        # ones columns (bf16) for the v-tile softmax-denominator cols
        ones_v = const.tile([128, NSB * HPC], BF16, tag="ones_v")
        nc.gpsimd.memset(ones_v[:], 1.0)
        # bf16 ones for the recip partition-broadcast matmul; row 64 is used
        # so its base partition matches the denominator row of the acc tiles
        ones_bc = const.tile([65, 64], BF16, tag="ones_bc")
        nc.gpsimd.memset(ones_bc[:], 1.0)
        # causal mask for diagonal 128x128 blocks of S^T[k, q]: keep q >= k
        mask_f = const.tile([128, 128], F32, tag="mask_f")
        nc.gpsimd.memset(mask_f[:], 1.0)
        nc.gpsimd.affine_select(
            mask_f[:], mask_f[:], pattern=[[1, 128]],
            compare_op=mybir.AluOpType.is_ge, fill=0.0,
            base=0, channel_multiplier=-1,
        )
        mask = const.tile([128, 128], BF16, tag="mask")
        nc.vector.tensor_copy(mask[:], mask_f[:])

        # --- x^T first (the v matmuls need it before anything else) ------
        xt = xpool.tile([128, NDB * S], BF16, tag="xt")
        nc.sync.dma_start(
            xt[:].rearrange("p (a s) -> p a s", s=S),
            xt_d.rearrange("(a p) s -> p a s", p=128),
        )

        # --- weights / biases (v first; Wo last, only o_proj needs it) ---
        w_t = {}
        for name, wd in (("v", wv_d), ("q", wq_d), ("k", wk_d)):
            t = wpool.tile([128, NDB * J], BF16, tag=f"w{name}")
            nc.sync.dma_start(
                t[:].rearrange("p (a j) -> p a j", j=J),
                wd.rearrange("(a p) j -> p a j", p=128),
            )
            w_t[name] = t
        bqt = const.tile([128, NJB], F32, tag="bq")
        nc.sync.dma_start(bqt[:], bq_d)
        bkt = const.tile([128, NJB], F32, tag="bk")
        nc.sync.dma_start(bkt[:], bk_d)
        wo_t = wpool.tile([128, NJB * D], BF16, tag="wo")
        nc.sync.dma_start(
            wo_t[:].rearrange("p (a o) -> p a o", o=D),
            wo_d.rearrange("(a p) o -> p a o", p=128),
        )

        # --- v natural [s, j] with ones cols (16 s-block groups) ---------
        # v_t cols: s-block sb at [sb*8*VW, ...), head h at 65h..65h+64,
        # ones at 65h+64.
        v_t = vpool.tile([128, NSB * HPC * VW], BF16, tag="vt")
        nc.vector.tensor_copy(
            v_t[:].rearrange("p (sb h c) -> p (sb h) c", sb=NSB, c=VW)[:, :, 64:65],
            ones_v[:].rearrange("p (a o) -> p a o", o=1),
        )
        for sb in range(NSB):
            pv = ps.tile([128, 1024], F32, tag="ps", name="pv")
            for db in range(NDB):
                nc.tensor.matmul(
                    pv[:, 0:J],
                    xt[:, 2048 * db + 128 * sb: 2048 * db + 128 * (sb + 1)],
                    w_t["v"][:, J * db: J * (db + 1)],
                    start=(db == 0), stop=(db == NDB - 1),
                )
            nc.scalar.activation(
                v_t[:, sb * HPC * VW: (sb + 1) * HPC * VW].rearrange(
                    "p (h c) -> p h c", c=VW)[:, :, 0:64],
                pv[:, 0:J].rearrange("p (h c) -> p h c", c=64),
                IDENT,
            )

        # --- q^T / k^T [j, s] --------------------------------------------
        q_t = qkpool.tile([128, NJB * S], BF16, tag="qt")
        k_t = qkpool.tile([128, NJB * S], BF16, tag="kt")
        for jb in range(NJB):
            for name, dst, bias in (("q", q_t, bqt), ("k", k_t, bkt)):
                for c in range(NC):
                    pq = ps.tile([128, 1024], F32, tag="ps", name="pq")
                    for db in range(NDB):
                        nc.tensor.matmul(
                            pq[:, 0:512],
                            w_t[name][:, J * db + 128 * jb: J * db + 128 * (jb + 1)],
                            xt[:, 2048 * db + 512 * c: 2048 * db + 512 * (c + 1)],
                            start=(db == 0), stop=(db == NDB - 1),
                        )
                    nc.scalar.activation(
                        dst[:, S * jb + 512 * c: S * jb + 512 * (c + 1)],
                        pq[:, 0:512],
                        IDENT,
                        bias=bias[:, jb: jb + 1],
                    )

        # --- attention: 4 head-pairs (j-tiles), 4 q-chunks each ----------
        # Per (pair, chunk, kb): one combined score tile holds head A in cols
        # 0:512 and head B in 512:1024 (separate PSUM banks -> the two K=64
        # matmuls run on disjoint PE row-groups concurrently); one 2D-AP exp
        # covers both heads; AV accumulates per head into [65, 512] accs.
        aot = aopool.tile([128, NJB * S], BF16, tag="aot")
        rows = {"A": slice(0, 64), "B": slice(64, 128)}
        for t in range(NJB):
            head = {"A": 2 * t, "B": 2 * t + 1}
            for c in range(NC):
                acc = {
                    X: ps_acc.tile([65, 512], F32, tag="acc", name=f"acc{X}")
                    for X in ("A", "B")
                }
                n_kb = 4 * c + 4

                def emit_av(pt, kb, wlo):
                    for X in ("A", "B"):
                        nc.tensor.matmul(
                            acc[X][0:65, wlo:512],
                            v_t[:, (8 * kb + head[X]) * VW:
                                (8 * kb + head[X]) * VW + VW],
                            pt[:, 512 * (X == "B") + wlo:
                               512 * (X == "B") + 512],
                            start=(kb == 0), stop=(kb == n_kb - 1),
                        )

                # software-pipelined: AV(kb-1) is emitted AFTER scores(kb) so
                # the in-order PE queue streams scores while ACT runs the exp
                # (an AV waiting on its exp would otherwise block the queue)
                pend = None
                for kb in range(n_kb):
                    wlo = max(0, 128 * kb - 512 * c)
                    st = ps.tile([128, 1024], F32, tag="ps", name="st")
                    for X in ("A", "B"):
                        nc.tensor.matmul(
                            st[:, 512 * (X == "B") + wlo:
                               512 * (X == "B") + 512],
                            k_t[rows[X],
                                S * t + 128 * kb: S * t + 128 * (kb + 1)],
                            q_t[rows[X],
                                S * t + 512 * c + wlo: S * t + 512 * (c + 1)],
                            start=True, stop=True,
                        )
                    pt = ptpool.tile([128, 1024], BF16, tag="pt", name="pt")
                    nc.scalar.activation(
                        pt[:].rearrange("p (x q) -> p x q", x=2)[:, :, wlo:512],
                        st[:].rearrange("p (x q) -> p x q", x=2)[:, :, wlo:512],
                        EXP, scale=0.125,
                    )
                    # causal mask on the diagonal 128x128 block: keep q >= k
                    if kb >= 4 * c:
                        for X in ("A", "B"):
                            o = 512 * (X == "B") + wlo
                            nc.vector.tensor_mul(
                                pt[:, o: o + 128], pt[:, o: o + 128], mask[:],
                            )
                    if pend is not None:
                        emit_av(*pend)
                    pend = (pt, kb, wlo)
                emit_av(*pend)
                # normalize: rows 0..63 raw attnout^T, row 64 = denominator.
                # Broadcast den to partitions 0..63 via a K=1 ones matmul,
                # then reciprocal at base partition 0 (reciprocal_approx_fast
                # mishandles PSUM sources and nonzero base partitions).
                # Head B lands at aot partitions 64..127 via an SBUF->SBUF
                # DMA partition shift (DVE lanes can't cross partitions).
                for X, off in (("A", 0), ("B", 64)):
                    a = acc[X]
                    dnb = nrmpool.tile([65, 512], BF16, tag="dnb")
                    nc.vector.tensor_copy(dnb[64:65, :], a[64:65, :])
                    bcd = ps.tile([64, 512], F32, tag="ps", name="bcd")
                    nc.tensor.matmul(
                        bcd[:], ones_bc[64:65, :], dnb[64:65, :],
                        start=True, stop=True,
                    )
                    dnf = nrmpool.tile([64, 512], F32, tag="dnf")
                    nc.vector.tensor_copy(dnf[:], bcd[:])
                    bct = nrmpool.tile([64, 512], F32, tag="bct")
                    nc.vector.reciprocal_approx_fast(bct[:], dnf[:])
                    if off == 0:
                        nc.vector.tensor_mul(
                            aot[0:64,
                                S * t + 512 * c: S * t + 512 * (c + 1)],
                            a[0:64, :], bct[:],
                        )
                    else:
                        tmp = nrmpool.tile([64, 512], BF16, tag="tmp")
                        nc.vector.tensor_mul(tmp[:], a[0:64, :], bct[:])
                        nc.sync.dma_start(
                            aot[64:128,
                                S * t + 512 * c: S * t + 512 * (c + 1)],
                            tmp[:],
                        )

        if dumps is not None:
            nc.sync.dma_start(dumps["d_qt"][:, :], q_t[:])
            nc.sync.dma_start(dumps["d_kt"][:, :], k_t[:])
            nc.sync.dma_start(dumps["d_vt"][:, :], v_t[:])
            nc.sync.dma_start(dumps["d_aot"][:, :], aot[:])

        # --- o_proj: out^T[o, s] partial = Wo_slice^T @ aot ---------------
        for ob in range(NDB):
            og = ogpool.tile([128, S], F32, tag="og")
            for c in range(NC):
                po = ps.tile([128, 1024], F32, tag="ps", name="po")
                for jb in range(NJB):
                    nc.tensor.matmul(
                        po[:, 0:512],
                        wo_t[:, D * jb + 128 * ob: D * jb + 128 * (ob + 1)],
                        aot[:, S * jb + 512 * c: S * jb + 512 * (c + 1)],
                        start=(jb == 0), stop=(jb == NJB - 1),
                    )
                if c % 2 == 0:
                    nc.scalar.copy(og[:, 512 * c: 512 * (c + 1)], po[:, 0:512])
                else:
                    nc.vector.tensor_copy(og[:, 512 * c: 512 * (c + 1)], po[:, 0:512])
            nc.sync.dma_start(out_d[128 * ob: 128 * (ob + 1), :], og[:])


_NC_CACHE = None


def _get_nc():
    global _NC_CACHE
    if _NC_CACHE is None:
        _NC_CACHE = build_kernel()
    return _NC_CACHE


def build_in_maps(inputs):
    """Host-side sharding: per-core input dict for run_bass_kernel_spmd."""
    x = np.asarray(inputs["hidden_states"], np.float32)
    xt_b = [
        np.ascontiguousarray(x[b].T.astype(BF16_NP)) for b in range(B)
    ]  # [D, S] bf16 per batch
    Wq = np.asarray(inputs["Wq"], np.float32)
    Wk = np.asarray(inputs["Wk"], np.float32)
    Wv = np.asarray(inputs["Wv"], np.float32)
    Wo = np.asarray(inputs["Wo"], np.float32)
    bq = np.asarray(inputs["bq"], np.float32)
    bk = np.asarray(inputs["bk"], np.float32)

    in_maps = []
    for c in range(N_CORES):
        b, g = c // 2, c % 2
        js = slice(g * J, (g + 1) * J)
        in_maps.append({
            "xt": xt_b[b],
            "wq": np.ascontiguousarray(Wq[:, js].astype(BF16_NP)),
            "wk": np.ascontiguousarray(Wk[:, js].astype(BF16_NP)),
            "wv": np.ascontiguousarray(Wv[:, js].astype(BF16_NP)),
            "wo": np.ascontiguousarray(Wo[js, :].astype(BF16_NP)),
            "bq": np.ascontiguousarray(bq[js].reshape(NJB, 128).T),
            "bk": np.ascontiguousarray(bk[js].reshape(NJB, 128).T),
        })
    return in_maps


def assemble_output(results, inputs):
    """Sum per-batch partial pairs, transpose, add folded bias."""
    Wo = np.asarray(inputs["Wo"], np.float32)
    bv = np.asarray(inputs["bv"], np.float32)
    bo = np.asarray(inputs["bo"], np.float32)
    bo_f = bv @ Wo + bo
    out = np.empty((B, S, D), np.float32)
    for b in range(B):
        acc = results[2 * b]["out_t"].astype(np.float32) + \
            results[2 * b + 1]["out_t"].astype(np.float32)
        out[b] = acc.T + bo_f[None, :]
    return out


def kernel(**inputs) -> np.ndarray:
    nc = _get_nc()
    in_maps = build_in_maps(inputs)
    res = run_bass_kernel_spmd(nc, in_maps, core_ids=list(range(N_CORES)))
    return assemble_output(res.results, inputs)


if __name__ == "__main__":
    rng = np.random.default_rng(0)
    ins = {
        "hidden_states": rng.standard_normal((B, S, D)).astype(np.float32),
        "Wq": (rng.standard_normal((D, D)) * 0.02).astype(np.float32),
        "bq": np.zeros(D, np.float32),
        "Wk": (rng.standard_normal((D, D)) * 0.02).astype(np.float32),
        "bk": np.zeros(D, np.float32),
        "Wv": (rng.standard_normal((D, D)) * 0.02).astype(np.float32),
        "bv": np.zeros(D, np.float32),
        "Wo": (rng.standard_normal((D, D)) * 0.02).astype(np.float32),
        "bo": np.zeros(D, np.float32),
    }
    out = kernel(**ins)
    print("out", out.shape, out.dtype, float(np.abs(out).mean()))
